# revision 13
# baseline (speedup 1.0000x reference)
"""Trainium2 Bass kernel for CompositionalAttentionBase.

Problem (per batch element b, reference semantics):
  q = (x @ Wq + bq)  -> [T,H,P] * 1/sqrt(P)
  k = (x @ Wk + bk)  -> [T,H,P]
  v = (x @ Wv + bv)  -> [T,H,R,P]
  score = softmax(q k^T) per head            [H,Tq,Tk]
  out   = score @ v per (head, rule)         [T,H,R,P]
  q_v = (x @ Wqv + bqv)/sqrt(QK)             [T,H,QK]
  k_v = out @ Wkv + bkv                      [T,H,R,QK]
  comp = softmax_r(q_v . k_v)                [T,H,R]
  out2 = sum_r comp * out                    [T,H,P]
  y = out2.reshape(T,D) @ Wm

Sharding: pure data-parallel over batch. B == n_cores == 8, so each
NeuronCore computes one full batch element; no collectives at all.

v2 design notes (vs the fp32r baseline):
  - Everything on the PE is bf16 (PSUM accumulation stays fp32). At
    N=512 the matmul streams at 1 col/cycle for both fp32r and bf16,
    but bf16 enables fast-weight-load (64-cycle LDWEIGHTS, hidden
    behind the 512-cycle matmul) and halves all SBUF/DMA/evacuation
    traffic.
  - x is pre-transposed on the host (xT [D,T]) and all weights are
    pre-packed host-side into per-head contiguous layouts, so every
    weight load is one large DMA and the kernel does zero PE
    transposes.
  - The per-head program is split into stage A (projections, scores,
    V, retrieval OTu, composition logits) and stage B (composition
    softmax tail + rule-weighted sum). B(h-1) is emitted after A(h),
    so the PE never waits on the vector-engine softmax chain at a
    head boundary.
  - The rule-weighted sum uses a contiguous multiply/add chain on
    DVE instead of one big strided tensor_reduce (which measured
    ~15us per head in the baseline trace).

Per-core dataflow (head-by-head; all contractions natural TensorE
matmuls, scores kept unnormalized with 1/Z folded into the final
composition weights):
  qT_h = Wq_h^T @ xT        [P,T]     (Wq pre-scaled by 1/sqrt(P))
  kT_h = Wk_h^T @ xT        [P,T]
  ET   = exp(kT^T q-slices) [Tk,Tq]
  V_h  = xT^T @ Wv_h        [Tk,R*P]
  OTu_r = V_r^T @ ET        [P,Tq]    (unnormalized attention out)
  ZRep8 = ones8^T @ ET      [8,Tq]    -> recipZ
  qvT  = Wqv_h^T @ xT (4x row-replicated) [4*QK,T]
  kvT  = Wkv^T @ OTu_r (block-diag, 4 rules/psum) [4*QK,Tq]
  compU = blockdiag-sums of (kvT * qvT)   [8,Tq]
  compE = exp(compU / Z);  w = compE / (CZ * Z)
  out2_h = sum_r OTu_r * broadcast(w_r)   [P,T]
  y = sum_h out2_h^T @ Wm_h               [T,D]
"""

import numpy as np
import ml_dtypes

import concourse.bass as bass
import concourse.tile as tile
from concourse import bacc, mybir
from concourse.bass_utils import run_bass_kernel_spmd

B, T, D, H, R, QK = 8, 1024, 1024, 8, 8, 32
P = D // H  # 128
NCORES = 8
TT = T // 128  # 8 t-tiles
KT = D // 128  # 8 contraction tiles for D
NC2 = T // 512  # 2 free-dim chunks of 512 over T
F32 = mybir.dt.float32
BF16 = mybir.dt.bfloat16
EXP = mybir.ActivationFunctionType.Exp
MUL = mybir.AluOpType.mult
ADD = mybir.AluOpType.add


def _c(c):  # 512-chunk slice
    return slice(c * 512, (c + 1) * 512)


def _t(i):  # 128-tile slice
    return slice(i * 128, (i + 1) * 128)


def build_kernel(tc, io, flags):
    nc = tc.nc

    with (
        nc.allow_low_precision(reason="bf16 intermediates; end-to-end precision validated vs reference"),
        tc.tile_pool(name="cst", bufs=1) as cst,
        tc.tile_pool(name="per", bufs=1) as per,
        tc.tile_pool(name="hd", bufs=2) as hd,     # double-buffered per-head
        tc.tile_pool(name="hs", bufs=1) as hs,     # single-buffered per-head
        tc.tile_pool(name="sc", bufs=2) as scp,    # small vector scratch
        tc.tile_pool(name="psA", bufs=6, space="PSUM") as psA,
        tc.tile_pool(name="psS", bufs=2, space="PSUM") as psS,
    ):
        # ---- constants (DMAs emitted after the critical first-head
        # weight loads; none is read before the Z stage of head 0) ----
        ones8 = cst.tile([128, 8], BF16, name="ones8")
        blkA = cst.tile([128, 8], BF16, name="blkA")
        blkB = cst.tile([128, 8], BF16, name="blkB")
        ones88 = cst.tile([8, 8], BF16, name="ones88")
        sel8 = cst.tile([8, 1024], BF16, name="sel8")
        wkvblk = cst.tile([128, 4, 128], BF16, name="wkvblk")

        def load_consts():
            nc.sync.dma_start(ones8[:], io["c_ones8"])
            nc.sync.dma_start(blkA[:], io["c_blkA"])
            nc.sync.dma_start(blkB[:], io["c_blkB"])
            nc.sync.dma_start(ones88[:], io["c_ones88"])
            nc.sync.dma_start(sel8[:], io["c_sel8"])
            nc.sync.dma_start(wkvblk[:], io["c_wkvblk"].rearrange("p (g m) -> p g m", g=4))
        if flags["bq"]:
            bq_sb = cst.tile([128, 8], F32, name="bq_sb")
            nc.sync.dma_start(bq_sb[:], io["bq"].rearrange("(h p) -> p h", p=128))
        if flags["bk"]:
            bk_sb = cst.tile([128, 8], F32, name="bk_sb")
            nc.sync.dma_start(bk_sb[:], io["bk"].rearrange("(h p) -> p h", p=128))
        if flags["bqv"]:
            bqv_sb = cst.tile([128, 8], F32, name="bqv_sb")
            nc.sync.dma_start(
                bqv_sb[:],
                io["bqv"].rearrange("(h q) -> q h", q=32).to_broadcast([4, 32, 8]).rearrange("r q h -> (r q) h"),
            )
        if flags["bv"]:
            onesrow = cst.tile([1, 128], BF16, name="onesrow")
            nc.sync.dma_start(onesrow[:], io["c_onesrow"])
        if flags["bkv"]:
            bkv_sb = cst.tile([128, 1], F32, name="bkv_sb")
            nc.sync.dma_start(
                bkv_sb[:], io["bkv"].rearrange("(o q) -> q o", o=1).to_broadcast([4, 32, 1]).rearrange("r q o -> (r q) o")
            )

        # ---- persistent tiles (wm DMA deferred; see head loop) ----
        xT = per.tile([128, KT, T], BF16, name="xT")
        wm = per.tile([128, H, D], BF16, name="wm")
        out2 = per.tile([128, H, T], BF16, name="out2")

        # ---- per-head weight loads (prefetched one head ahead) ----
        wq = [None] * H
        wk = [None] * H
        wqv = [None] * H
        wv = [None] * H

        def load_weights(h):
            wq[h] = hd.tile([128, D], BF16, tag="wq", name=f"wq{h}")
            nc.sync.dma_start(wq[h][:], io["WqP"][h])
            wk[h] = hd.tile([128, D], BF16, tag="wk", name=f"wk{h}")
            nc.sync.dma_start(wk[h][:], io["WkP"][h])
            wqv[h] = hd.tile([128, D], BF16, tag="wqv", name=f"wqv{h}")
            nc.sync.dma_start(wqv[h][:], io["WqvP"][h])
            wv[h] = hd.tile([128, KT, 1024], BF16, tag="wv", name=f"wv{h}", bufs=1)
            nc.sync.dma_start(wv[h][:], io["WvP"][h].rearrange("k (a rp) -> k a rp", a=KT))

        # per-head state handed between stages
        OTu_t = [None] * H
        recipZ_t = [None] * H
        compE_t = [None] * H
        ET_t = [None] * H
        V_t = [None] * H

        def stage_a(h, after_qk=None, step_cb=None):
            # ---- qT / kT (one 2-bank psum + one 1024-wide evac each) ----
            qT = hd.tile([128, T], BF16, tag="qT", name=f"qT{h}", bufs=1)
            kT = hd.tile([128, T], BF16, tag="kT", name=f"kT{h}", bufs=1)
            for dst, w, bflag in (
                (qT, wq[h], flags["bq"]),
                (kT, wk[h], flags["bk"]),
            ):
                for c in range(NC2):
                    ps = psA.tile([128, 512], F32, tag="acc", name=f"psqk{h}_{c}")
                    for kt in range(KT):
                        nc.tensor.matmul(
                            ps[:], w[:, _t(kt)], xT[:, kt, _c(c)],
                            start=(kt == 0), stop=(kt == KT - 1),
                        )
                    if bflag:
                        nc.scalar.activation(
                            dst[:, _c(c)], ps[:],
                            mybir.ActivationFunctionType.Identity,
                            bias=(bq_sb if dst is qT else bk_sb)[:, h : h + 1],
                        )
                    else:
                        nc.scalar.copy(dst[:, _c(c)], ps[:])

            if after_qk is not None:
                after_qk()

            # ---- ET = exp(scores^T) interleaved with V so the PE covers
            # the scalar-engine exp evacuations ----
            ET = hs.tile([128, TT, T], BF16, tag="ET", name=f"ET{h}")
            V = hs.tile([128, TT, 1024], BF16, tag="V", name=f"V{h}")
            ET_t[h], V_t[h] = ET, V
            for i in range(TT):
                # ET group (tk=i): 2 matmuls + exps
                for c in range(NC2):
                    pe = psA.tile([128, 512], F32, tag="acc", name=f"pse{h}_{i}_{c}")
                    nc.tensor.matmul(pe[:], kT[:, _t(i)], qT[:, _c(c)], start=True, stop=True)
                    nc.scalar.activation(ET[:, i, _c(c)], pe[:], EXP)
                # V group (tt=i)
                for c in range(2):
                    pv = psA.tile([128, 512], F32, tag="acc", name=f"psv{h}_{i}_{c}")
                    for kt in range(KT):
                        nc.tensor.matmul(
                            pv[:], xT[:, kt, _t(i)], wv[h][:, kt, _c(c)],
                            start=(kt == 0), stop=(kt == KT - 1 and not flags["bv"]),
                        )
                    if flags["bv"]:
                        if c == 0:
                            bv_t = scp.tile([1, 1024], BF16, tag="bv", name=f"bv{h}_{i}")
                            nc.sync.dma_start(bv_t[:], io["bv"][None, h * 1024 : (h + 1) * 1024])
                        nc.tensor.matmul(pv[:], onesrow[:], bv_t[:, _c(c)], start=False, stop=True)
                    nc.vector.tensor_copy(V[:, i, _c(c)], pv[:])
                if step_cb is not None:
                    step_cb(i)

            # post-V per-chunk work is emitted by the head loop (post_v)

        qvRep_t = [None] * H
        PP_t = [None] * H

        def post_v(h, c, pair_cb=None):
            """OTu/Z/qv/kv/compU for one 512-wide t-chunk, with optional
            B2-pair hooks interleaved between OTu rule tiles."""
            ET, V = ET_t[h], V_t[h]
            if c == 0:
                OTu_t[h] = hd.tile([128, R, T], BF16, tag="OTu", name=f"OTu{h}")
                recipZ_t[h] = hd.tile([8, T], F32, tag="recipZ", name=f"recipZ{h}")
                qvRep_t[h] = hs.tile([128, T], BF16, tag="qvRep", name=f"qvRep{h}")
                PP_t[h] = hs.tile([128, 2, T], BF16, tag="PP", name=f"PP{h}")
                compE_t[h] = hd.tile([8, T], BF16, tag="compE", name=f"compE{h}")
            OTu, recipZ, qvRep, PP, compE = (
                OTu_t[h], recipZ_t[h], qvRep_t[h], PP_t[h], compE_t[h])

            # ---- OTu_r = V_r^T @ ET  [128(p), R, T(q)] ----
            for r in range(R):
                po = psA.tile([128, 512], F32, tag="acc", name=f"pso{h}_{r}_{c}")
                for tk in range(TT):
                    nc.tensor.matmul(
                        po[:], V[:, tk, _t(r)], ET[:, tk, _c(c)],
                        start=(tk == 0), stop=(tk == TT - 1),
                    )
                nc.scalar.copy(OTu[:, r, _c(c)], po[:])
                if pair_cb is not None and r % 2 == 1:
                    pair_cb((r - 1) // 2)

            # ---- Z (softmax denominator) -> recipZ ----
            pz = psS.tile([8, 512], F32, tag="small", name=f"psz{h}_{c}")
            for tk in range(TT):
                nc.tensor.matmul(
                    pz[:], ones8[:], ET[:, tk, _c(c)],
                    start=(tk == 0), stop=(tk == TT - 1),
                )
            nc.vector.reciprocal_approx_fast(recipZ[:, _c(c)], pz[:])

            # ---- qvRep [128(4x qk), T] ----
            pq = psA.tile([128, 512], F32, tag="acc", name=f"psq{h}_{c}")
            for kt in range(KT):
                nc.tensor.matmul(
                    pq[:], wqv[h][:, _t(kt)], xT[:, kt, _c(c)],
                    start=(kt == 0), stop=(kt == KT - 1),
                )
            if flags["bqv"]:
                nc.scalar.activation(
                    qvRep[:, _c(c)], pq[:],
                    mybir.ActivationFunctionType.Identity,
                    bias=bqv_sb[:, h : h + 1],
                )
            else:
                nc.scalar.copy(qvRep[:, _c(c)], pq[:])

            # ---- kvT (4 rules / psum via block-diag Wkv) + P-mul ----
            for g in range(2):
                pk = psA.tile([128, 512], F32, tag="acc", name=f"psk{h}_{c}_{g}")
                for rr in range(4):
                    r = g * 4 + rr
                    nc.tensor.matmul(
                        pk[:], wkvblk[:, rr], OTu[:, r, _c(c)],
                        start=(rr == 0), stop=(rr == 3),
                    )
                if flags["bkv"]:
                    tmp = scp.tile([128, 512], F32, tag="kvtmp", name=f"kvt{h}_{c}_{g}")
                    nc.vector.tensor_scalar_add(tmp[:], pk[:], bkv_sb[:, 0:1])
                    nc.vector.tensor_tensor(PP[:, g, _c(c)], tmp[:], qvRep[:, _c(c)], op=MUL)
                else:
                    nc.vector.tensor_tensor(PP[:, g, _c(c)], pk[:], qvRep[:, _c(c)], op=MUL)

            # ---- compU -> comp logits -> compE ----
            pc = psS.tile([8, 512], F32, tag="small", name=f"psc{h}_{c}")
            nc.tensor.matmul(pc[:], blkA[:], PP[:, 0, _c(c)], start=True, stop=False)
            nc.tensor.matmul(pc[:], blkB[:], PP[:, 1, _c(c)], start=False, stop=True)
            compL = scp.tile([8, 512], BF16, tag="compL", name=f"compL{h}_{c}", bufs=1)
            nc.vector.tensor_tensor(compL[:], pc[:], recipZ[:, _c(c)], op=MUL)
            nc.scalar.activation(compE[:, _c(c)], compL[:], EXP)

        w8_t = [None] * H
        w8_t = [None] * H

        def stage_b1c(h, c):
            recipZ, compE = recipZ_t[h], compE_t[h]
            # ---- CZ -> w8 = compE / (CZ * Z) ----
            if c == 0:
                w8_t[h] = hs.tile([8, T], BF16, tag="w8", name=f"w8{h}")
            w8 = w8_t[h]
            if True:
                pcz = psS.tile([8, 512], F32, tag="small", name=f"pscz{h}_{c}")
                nc.tensor.matmul(pcz[:], ones88[:], compE[:, _c(c)], start=True, stop=True)
                recipCZ = scp.tile([8, 512], F32, tag="recipCZ", name=f"rcz{h}_{c}", bufs=1)
                nc.vector.reciprocal_approx_fast(recipCZ[:], pcz[:])
                denom = scp.tile([8, 512], BF16, tag="denom", name=f"den{h}_{c}", bufs=1)
                nc.vector.tensor_tensor(denom[:], recipCZ[:], recipZ[:, _c(c)], op=MUL)
                nc.vector.tensor_tensor(w8[:, _c(c)], compE[:, _c(c)], denom[:], op=MUL)

        def stage_b1(h):
            stage_b1c(h, 0)
            stage_b1c(h, 1)

        b2_acc = {}

        def stage_b2_pair(h, c, pair):
            # ---- broadcast w (PE select-matmul, rule pairs share a
            # 2-bank psum tile) + weighted sum over rules; one pair of
            # rules per call so the vector chain spreads across the
            # next head's stage-A hooks ----
            OTu, w8 = OTu_t[h], w8_t[h]
            for half in range(2):
                r = 2 * pair + half
                wr_tile = psA.tile([128, 512], F32, tag="acc", name=f"wrps{h}_{c}_{r}")
                wr = wr_tile[:]
                nc.tensor.matmul(wr, sel8[:, _t(r)], w8[:, _c(c)], start=True, stop=True)
                if r == 0:
                    acc = scp.tile([128, 512], BF16, tag=f"acc{c}a", name=f"ac{h}_{c}_0", bufs=1)
                    nc.vector.tensor_tensor(acc[:], wr, OTu[:, r, _c(c)], op=MUL)
                    b2_acc[(h, c)] = acc
                else:
                    acc = b2_acc[(h, c)]
                    prod = scp.tile([128, 512], BF16, tag=f"prod{c}", name=f"pr{h}_{c}_{r}")
                    nc.vector.tensor_tensor(prod[:], wr, OTu[:, r, _c(c)], op=MUL)
                    if r < R - 1:
                        nacc = scp.tile([128, 512], BF16, tag=f"acc{c}{'b' if r % 2 else 'a'}", name=f"ac{h}_{c}_{r}", bufs=1)
                        nc.vector.tensor_tensor(nacc[:], acc[:], prod[:], op=ADD)
                        b2_acc[(h, c)] = nacc
                    else:
                        nc.vector.tensor_tensor(out2[:, h, _c(c)], acc[:], prod[:], op=ADD)

        def stage_b2(h, c):
            for pair in range(4):
                stage_b2_pair(h, c, pair)

        def merge_tiles(tts, pair_cb=None):
            # y = sum_h out2_h^T @ Wm_h for the given t-tiles
            for idx, tt in enumerate(tts):
                if pair_cb is not None and idx < 4:
                    pair_cb(idx)
                for c in range(NC2):
                    py = psA.tile([128, 512], F32, tag="acc", name=f"psy{tt}_{c}")
                    for h in range(H):
                        nc.tensor.matmul(
                            py[:], out2[:, h, _t(tt)], wm[:, h, _c(c)],
                            start=(h == 0), stop=(h == H - 1),
                        )
                    yt = scp.tile([128, 512], F32, tag="yt", name=f"yt{tt}_{c}")
                    nc.scalar.copy(yt[:], py[:])
                    nc.sync.dma_start(io["y"][_t(tt), _c(c)], yt[:])

        # ---- software-pipelined head loop: the previous head's
        # composition tail (B1 = w8 chain, B2 = broadcast + weighted
        # sum) is emitted interleaved into this head's stage A so the
        # PE never throttles on the vector chain. The startup DMA order
        # is critical-path aware (Sync triggers serialize at ~0.6us
        # each): wq first, then xT, then the rest of head 0's weights;
        # constants after those; wm (merge weights) deferred; later
        # heads prefetch from inside the previous head's stage A.
        wq[0] = hd.tile([128, D], BF16, tag="wq", name="wq0")
        nc.sync.dma_start(wq[0][:], io["WqP"][0])
        for kt in range(KT):
            nc.sync.dma_start(xT[:, kt], io["xT"][_t(kt), :])
        wk[0] = hd.tile([128, D], BF16, tag="wk", name="wk0")
        nc.sync.dma_start(wk[0][:], io["WkP"][0])
        wqv[0] = hd.tile([128, D], BF16, tag="wqv", name="wqv0")
        nc.sync.dma_start(wqv[0][:], io["WqvP"][0])
        wv[0] = hd.tile([128, KT, 1024], BF16, tag="wv", name="wv0", bufs=1)
        nc.sync.dma_start(wv[0][:], io["WvP"][0].rearrange("k (a rp) -> k a rp", a=KT))
        load_consts()

        def prefetch(h):
            if h == 2:
                nc.sync.dma_start(wm[:], io["WmP"].rearrange("k (h d) -> k h d", h=H))
            if h < H:
                load_weights(h)

        for h in range(H):
            if h >= 1:
                stage_a(
                    h,
                    after_qk=lambda hh=h - 1: stage_b1(hh),
                    step_cb=lambda i, hh=h - 1: stage_b2_pair(hh, i // 4, i % 4),
                )
            else:
                stage_a(h)
            prefetch(h + 1)
            if h < H - 1:
                post_v(h, 0)
                post_v(h, 1)
            else:
                # last head: pipeline its own composition tail against
                # chunk-1 work and the merge so the PE never drains
                post_v(h, 0)
                stage_b1c(h, 0)
                post_v(h, 1, pair_cb=lambda p: stage_b2_pair(h, 0, p))
                stage_b1c(h, 1)
                merge_tiles(range(0, 4), pair_cb=lambda p: stage_b2_pair(h, 1, p))
                merge_tiles(range(4, 8))



_CACHE = {}


def _build(flags_key):
    if flags_key in _CACHE:
        return _CACHE[flags_key]
    flags = dict(flags_key)
    nc = bacc.Bacc("TRN2", target_bir_lowering=False, debug=False, num_devices=NCORES)
    io = {}
    io["xT"] = nc.dram_tensor("xT", [D, T], BF16, kind="ExternalInput").ap()
    io["WqP"] = nc.dram_tensor("WqP", [H, 128, D], BF16, kind="ExternalInput").ap()
    io["WkP"] = nc.dram_tensor("WkP", [H, 128, D], BF16, kind="ExternalInput").ap()
    io["WqvP"] = nc.dram_tensor("WqvP", [H, 128, D], BF16, kind="ExternalInput").ap()
    io["WvP"] = nc.dram_tensor("WvP", [H, 128, KT * 1024], BF16, kind="ExternalInput").ap()
    io["WmP"] = nc.dram_tensor("WmP", [128, H * D], BF16, kind="ExternalInput").ap()
    io["c_wkvblk"] = nc.dram_tensor("c_wkvblk", [128, 512], BF16, kind="ExternalInput").ap()
    for bname, shape in (
        ("bq", [D]), ("bk", [D]), ("bv", [H * R * P]), ("bqv", [H * QK]), ("bkv", [QK]),
    ):
        if flags[bname]:
            dt = BF16 if bname == "bv" else F32
            io[bname] = nc.dram_tensor(bname, shape, dt, kind="ExternalInput").ap()
    io["c_ones8"] = nc.dram_tensor("c_ones8", [128, 8], BF16, kind="ExternalInput").ap()
    io["c_blkA"] = nc.dram_tensor("c_blkA", [128, 8], BF16, kind="ExternalInput").ap()
    io["c_blkB"] = nc.dram_tensor("c_blkB", [128, 8], BF16, kind="ExternalInput").ap()
    io["c_ones88"] = nc.dram_tensor("c_ones88", [8, 8], BF16, kind="ExternalInput").ap()
    io["c_sel8"] = nc.dram_tensor("c_sel8", [8, 1024], BF16, kind="ExternalInput").ap()
    if flags["bv"]:
        io["c_onesrow"] = nc.dram_tensor("c_onesrow", [1, 128], BF16, kind="ExternalInput").ap()
    io["y"] = nc.dram_tensor("y", [T, D], F32, kind="ExternalOutput").ap()

    with tile.TileContext(nc) as tc:
        build_kernel(tc, io, flags)
    nc.compile()
    _CACHE[flags_key] = (nc, flags)
    return _CACHE[flags_key]


def _wkvblk(Wkv):
    blk = np.zeros((128, 4, 128), np.float32)
    for g in range(4):
        blk[:, g, g * 32 : (g + 1) * 32] = Wkv
    return np.ascontiguousarray(blk.reshape(128, 512))


def _consts():
    ones8 = np.ones((128, 8), np.float32)
    blkA = np.zeros((128, 8), np.float32)
    blkB = np.zeros((128, 8), np.float32)
    for k in range(128):
        g = k // 32
        blkA[k, g] = 1.0
        blkB[k, 4 + g] = 1.0
    ones88 = np.ones((8, 8), np.float32)
    onesrow = np.ones((1, 128), np.float32)
    sel8 = np.zeros((8, 1024), np.float32)
    for m in range(1024):
        sel8[m // 128, m] = 1.0
    return {
        "c_ones8": ones8, "c_blkA": blkA, "c_blkB": blkB,
        "c_ones88": ones88, "c_onesrow": onesrow, "c_sel8": sel8,
    }


def _bf(a):
    return np.ascontiguousarray(a.astype(ml_dtypes.bfloat16))


def _pack_base(inp, flags):
    scale_q = np.float32(1.0 / np.sqrt(P))
    scale_qv = np.float32(1.0 / np.sqrt(QK))
    Wq_s = inp["Wq"] * scale_q
    Wqv_s = inp["Wqv"] * scale_qv
    # WqP[h,k,kt*128+m] = Wq_s[kt*128+k, h*128+m]
    WqP = Wq_s.reshape(KT, 128, H, 128).transpose(2, 1, 0, 3).reshape(H, 128, D)
    WkP = inp["Wk"].reshape(KT, 128, H, 128).transpose(2, 1, 0, 3).reshape(H, 128, D)
    # WqvP[h,k,kt*128+rep*32+j] = Wqv_s[kt*128+k, h*32+j]
    A = Wqv_s.reshape(KT, 128, H, QK).transpose(2, 1, 0, 3)  # [H,128,KT,QK]
    WqvP = np.broadcast_to(A[:, :, :, None, :], (H, 128, KT, 4, QK)).reshape(H, 128, D)
    # WvP[h,k,kt*1024+rp] = Wv[kt*128+k, h*1024+rp]
    WvP = inp["Wv"].reshape(KT, 128, H, 1024).transpose(2, 1, 0, 3).reshape(H, 128, KT * 1024)
    # WmP[k, h*1024+d] = Wm[h*128+k, d]
    WmP = inp["Wm"].reshape(H, 128, D).transpose(1, 0, 2).reshape(128, H * D)
    consts = _consts()
    base = {
        "WqP": _bf(WqP), "WkP": _bf(WkP), "WqvP": _bf(WqvP),
        "WvP": _bf(WvP), "WmP": _bf(WmP),
        "c_wkvblk": _bf(_wkvblk(inp["Wkv"])),
        "c_ones8": _bf(consts["c_ones8"]), "c_blkA": _bf(consts["c_blkA"]),
        "c_blkB": _bf(consts["c_blkB"]), "c_ones88": _bf(consts["c_ones88"]),
        "c_sel8": _bf(consts["c_sel8"]),
    }
    if flags["bq"]:
        base["bq"] = np.ascontiguousarray(inp["bq"] * scale_q)
    if flags["bk"]:
        base["bk"] = np.ascontiguousarray(inp["bk"])
    if flags["bqv"]:
        base["bqv"] = np.ascontiguousarray(inp["bqv"] * scale_qv)
    if flags["bv"]:
        base["bv"] = _bf(inp["bv"])
        base["c_onesrow"] = _bf(consts["c_onesrow"])
    if flags["bkv"]:
        base["bkv"] = np.ascontiguousarray(inp["bkv"])
    return base


def _run(inputs, trace=False):
    inp = {k: np.ascontiguousarray(np.asarray(v, dtype=np.float32)) for k, v in inputs.items()}
    flags = {b: bool(np.any(inp[b])) for b in ("bq", "bk", "bv", "bqv", "bkv")}
    flags_key = tuple(sorted(flags.items()))
    nc, flags = _build(flags_key)
    base = _pack_base(inp, flags)
    in_maps = []
    for c in range(NCORES):
        m = dict(base)
        m["xT"] = _bf(inp["x"][c].T)
        in_maps.append(m)
    res = run_bass_kernel_spmd(nc, in_maps, list(range(NCORES)), trace=trace)
    out = np.stack([res.results[c]["y"] for c in range(NCORES)], axis=0)
    return out, res


def kernel(**inputs):
    out, _ = _run(inputs, trace=False)
    return out


def run_traced(inputs):
    """Like kernel() but with NTFF tracing; returns (out, BassKernelResults)."""
    return _run(inputs, trace=True)


# revision 16
# speedup vs baseline: 1.0058x; 1.0058x over previous
"""Trainium2 Bass kernel for CompositionalAttentionBase.

Problem (per batch element b, reference semantics):
  q = (x @ Wq + bq)  -> [T,H,P] * 1/sqrt(P)
  k = (x @ Wk + bk)  -> [T,H,P]
  v = (x @ Wv + bv)  -> [T,H,R,P]
  score = softmax(q k^T) per head            [H,Tq,Tk]
  out   = score @ v per (head, rule)         [T,H,R,P]
  q_v = (x @ Wqv + bqv)/sqrt(QK)             [T,H,QK]
  k_v = out @ Wkv + bkv                      [T,H,R,QK]
  comp = softmax_r(q_v . k_v)                [T,H,R]
  out2 = sum_r comp * out                    [T,H,P]
  y = out2.reshape(T,D) @ Wm

Sharding: pure data-parallel over batch. B == n_cores == 8, so each
NeuronCore computes one full batch element; no collectives at all.

v2 design notes (vs the fp32r baseline):
  - Everything on the PE is bf16 (PSUM accumulation stays fp32). At
    N=512 the matmul streams at 1 col/cycle for both fp32r and bf16,
    but bf16 enables fast-weight-load (64-cycle LDWEIGHTS, hidden
    behind the 512-cycle matmul) and halves all SBUF/DMA/evacuation
    traffic.
  - x is pre-transposed on the host (xT [D,T]) and all weights are
    pre-packed host-side into per-head contiguous layouts, so every
    weight load is one large DMA and the kernel does zero PE
    transposes.
  - The per-head program is split into stage A (projections, scores,
    V, retrieval OTu, composition logits) and stage B (composition
    softmax tail + rule-weighted sum). B(h-1) is emitted after A(h),
    so the PE never waits on the vector-engine softmax chain at a
    head boundary.
  - The rule-weighted sum uses a contiguous multiply/add chain on
    DVE instead of one big strided tensor_reduce (which measured
    ~15us per head in the baseline trace).

Per-core dataflow (head-by-head; all contractions natural TensorE
matmuls, scores kept unnormalized with 1/Z folded into the final
composition weights):
  qT_h = Wq_h^T @ xT        [P,T]     (Wq pre-scaled by 1/sqrt(P))
  kT_h = Wk_h^T @ xT        [P,T]
  ET   = exp(kT^T q-slices) [Tk,Tq]
  V_h  = xT^T @ Wv_h        [Tk,R*P]
  OTu_r = V_r^T @ ET        [P,Tq]    (unnormalized attention out)
  ZRep8 = ones8^T @ ET      [8,Tq]    -> recipZ
  qvT  = Wqv_h^T @ xT (4x row-replicated) [4*QK,T]
  kvT  = Wkv^T @ OTu_r (block-diag, 4 rules/psum) [4*QK,Tq]
  compU = blockdiag-sums of (kvT * qvT)   [8,Tq]
  compE = exp(compU / Z);  w = compE / (CZ * Z)
  out2_h = sum_r OTu_r * broadcast(w_r)   [P,T]
  y = sum_h out2_h^T @ Wm_h               [T,D]
"""

import numpy as np
import ml_dtypes

import concourse.bass as bass
import concourse.tile as tile
from concourse import bacc, mybir
from concourse.bass_utils import run_bass_kernel_spmd

B, T, D, H, R, QK = 8, 1024, 1024, 8, 8, 32
P = D // H  # 128
NCORES = 8
TT = T // 128  # 8 t-tiles
KT = D // 128  # 8 contraction tiles for D
NC2 = T // 512  # 2 free-dim chunks of 512 over T
F32 = mybir.dt.float32
BF16 = mybir.dt.bfloat16
EXP = mybir.ActivationFunctionType.Exp
MUL = mybir.AluOpType.mult
ADD = mybir.AluOpType.add


def _c(c):  # 512-chunk slice
    return slice(c * 512, (c + 1) * 512)


def _t(i):  # 128-tile slice
    return slice(i * 128, (i + 1) * 128)


def build_kernel(tc, io, flags):
    nc = tc.nc

    with (
        nc.allow_low_precision(reason="bf16 intermediates; end-to-end precision validated vs reference"),
        tc.tile_pool(name="cst", bufs=1) as cst,
        tc.tile_pool(name="per", bufs=1) as per,
        tc.tile_pool(name="hd", bufs=2) as hd,     # double-buffered per-head
        tc.tile_pool(name="hs", bufs=1) as hs,     # single-buffered per-head
        tc.tile_pool(name="sc", bufs=2) as scp,    # small vector scratch
        tc.tile_pool(name="w8d", bufs=2, space="DRAM") as w8dp,
        tc.tile_pool(name="psA", bufs=6, space="PSUM") as psA,
        tc.tile_pool(name="psS", bufs=2, space="PSUM") as psS,
    ):
        # ---- constants (DMAs emitted after the critical first-head
        # weight loads; none is read before the Z stage of head 0) ----
        ones8 = cst.tile([128, 8], BF16, name="ones8")
        blkA = cst.tile([128, 8], BF16, name="blkA")
        blkB = cst.tile([128, 8], BF16, name="blkB")
        ones88 = cst.tile([8, 8], BF16, name="ones88")
        sel8 = cst.tile([8, 1024], BF16, name="sel8")
        wkvblk = cst.tile([128, 4, 128], BF16, name="wkvblk")

        def load_consts():
            nc.sync.dma_start(ones8[:], io["c_ones8"])
            nc.sync.dma_start(blkA[:], io["c_blkA"])
            nc.sync.dma_start(blkB[:], io["c_blkB"])
            nc.sync.dma_start(ones88[:], io["c_ones88"])
            nc.sync.dma_start(sel8[:], io["c_sel8"])
            nc.sync.dma_start(wkvblk[:], io["c_wkvblk"].rearrange("p (g m) -> p g m", g=4))
        if flags["bq"]:
            bq_sb = cst.tile([128, 8], F32, name="bq_sb")
            nc.sync.dma_start(bq_sb[:], io["bq"].rearrange("(h p) -> p h", p=128))
        if flags["bk"]:
            bk_sb = cst.tile([128, 8], F32, name="bk_sb")
            nc.sync.dma_start(bk_sb[:], io["bk"].rearrange("(h p) -> p h", p=128))
        if flags["bqv"]:
            bqv_sb = cst.tile([128, 8], F32, name="bqv_sb")
            nc.sync.dma_start(
                bqv_sb[:],
                io["bqv"].rearrange("(h q) -> q h", q=32).to_broadcast([4, 32, 8]).rearrange("r q h -> (r q) h"),
            )
        if flags["bv"]:
            onesrow = cst.tile([1, 128], BF16, name="onesrow")
            nc.sync.dma_start(onesrow[:], io["c_onesrow"])
        if flags["bkv"]:
            bkv_sb = cst.tile([128, 1], F32, name="bkv_sb")
            nc.sync.dma_start(
                bkv_sb[:], io["bkv"].rearrange("(o q) -> q o", o=1).to_broadcast([4, 32, 1]).rearrange("r q o -> (r q) o")
            )

        # ---- persistent tiles (wm DMA deferred; see head loop) ----
        xT = per.tile([128, KT, T], BF16, name="xT")
        wm = per.tile([128, H, D], BF16, name="wm")
        out2 = per.tile([128, H, T], BF16, name="out2")

        # ---- per-head weight loads (prefetched one head ahead) ----
        wq = [None] * H
        wk = [None] * H
        wqv = [None] * H
        wv = [None] * H

        def load_weights(h):
            wq[h] = hd.tile([128, D], BF16, tag="wq", name=f"wq{h}")
            nc.sync.dma_start(wq[h][:], io["WqP"][h])
            wk[h] = hd.tile([128, D], BF16, tag="wk", name=f"wk{h}")
            nc.sync.dma_start(wk[h][:], io["WkP"][h])
            wqv[h] = hd.tile([128, D], BF16, tag="wqv", name=f"wqv{h}")
            nc.sync.dma_start(wqv[h][:], io["WqvP"][h])
            wv[h] = hd.tile([128, KT, 1024], BF16, tag="wv", name=f"wv{h}", bufs=1)
            nc.sync.dma_start(wv[h][:], io["WvP"][h].rearrange("k (a rp) -> k a rp", a=KT))

        # per-head state handed between stages
        OTu_t = [None] * H
        recipZ_t = [None] * H
        compE_t = [None] * H
        ET_t = [None] * H
        V_t = [None] * H

        def stage_a(h, after_qk=None, step_cb=None):
            # ---- qT / kT (one 2-bank psum + one 1024-wide evac each) ----
            qT = hd.tile([128, T], BF16, tag="qT", name=f"qT{h}", bufs=1)
            kT = hd.tile([128, T], BF16, tag="kT", name=f"kT{h}", bufs=1)
            for dst, w, bflag in (
                (qT, wq[h], flags["bq"]),
                (kT, wk[h], flags["bk"]),
            ):
                for c in range(NC2):
                    ps = psA.tile([128, 512], F32, tag="acc", name=f"psqk{h}_{c}")
                    for kt in range(KT):
                        nc.tensor.matmul(
                            ps[:], w[:, _t(kt)], xT[:, kt, _c(c)],
                            start=(kt == 0), stop=(kt == KT - 1),
                        )
                    if bflag:
                        nc.scalar.activation(
                            dst[:, _c(c)], ps[:],
                            mybir.ActivationFunctionType.Identity,
                            bias=(bq_sb if dst is qT else bk_sb)[:, h : h + 1],
                        )
                    else:
                        nc.scalar.copy(dst[:, _c(c)], ps[:])

            if after_qk is not None:
                after_qk()

            # ---- ET = exp(scores^T) interleaved with V so the PE covers
            # the scalar-engine exp evacuations ----
            ET = hs.tile([128, TT, T], BF16, tag="ET", name=f"ET{h}")
            V = hs.tile([128, TT, 1024], BF16, tag="V", name=f"V{h}")
            ET_t[h], V_t[h] = ET, V
            for i in range(TT):
                # ET group (tk=i): 2 matmuls + exps
                for c in range(NC2):
                    pe = psA.tile([128, 512], F32, tag="acc", name=f"pse{h}_{i}_{c}")
                    nc.tensor.matmul(pe[:], kT[:, _t(i)], qT[:, _c(c)], start=True, stop=True)
                    nc.scalar.activation(ET[:, i, _c(c)], pe[:], EXP)
                # V group (tt=i)
                for c in range(2):
                    pv = psA.tile([128, 512], F32, tag="acc", name=f"psv{h}_{i}_{c}")
                    for kt in range(KT):
                        nc.tensor.matmul(
                            pv[:], xT[:, kt, _t(i)], wv[h][:, kt, _c(c)],
                            start=(kt == 0), stop=(kt == KT - 1 and not flags["bv"]),
                        )
                    if flags["bv"]:
                        if c == 0:
                            bv_t = scp.tile([1, 1024], BF16, tag="bv", name=f"bv{h}_{i}")
                            nc.sync.dma_start(bv_t[:], io["bv"][None, h * 1024 : (h + 1) * 1024])
                        nc.tensor.matmul(pv[:], onesrow[:], bv_t[:, _c(c)], start=False, stop=True)
                    nc.vector.tensor_copy(V[:, i, _c(c)], pv[:])
                if step_cb is not None:
                    step_cb(i)

            # post-V per-chunk work is emitted by the head loop (post_v)

        qvRep_t = [None] * H
        PP_t = [None] * H

        def post_v(h, c, pair_cb=None):
            """OTu/Z/qv/kv/compU for one 512-wide t-chunk, with optional
            B2-pair hooks interleaved between OTu rule tiles."""
            ET, V = ET_t[h], V_t[h]
            if c == 0:
                OTu_t[h] = hd.tile([128, R, T], BF16, tag="OTu", name=f"OTu{h}")
                recipZ_t[h] = hd.tile([8, T], F32, tag="recipZ", name=f"recipZ{h}")
                qvRep_t[h] = hs.tile([128, T], BF16, tag="qvRep", name=f"qvRep{h}")
                PP_t[h] = hs.tile([128, 2, T], BF16, tag="PP", name=f"PP{h}")
                compE_t[h] = hd.tile([8, T], BF16, tag="compE", name=f"compE{h}")
            OTu, recipZ, qvRep, PP, compE = (
                OTu_t[h], recipZ_t[h], qvRep_t[h], PP_t[h], compE_t[h])

            # ---- OTu_r = V_r^T @ ET  [128(p), R, T(q)] ----
            for r in range(R):
                po = psA.tile([128, 512], F32, tag="acc", name=f"pso{h}_{r}_{c}")
                for tk in range(TT):
                    nc.tensor.matmul(
                        po[:], V[:, tk, _t(r)], ET[:, tk, _c(c)],
                        start=(tk == 0), stop=(tk == TT - 1),
                    )
                nc.scalar.copy(OTu[:, r, _c(c)], po[:])
                if pair_cb is not None and r % 2 == 1:
                    pair_cb((r - 1) // 2)

            # ---- Z (softmax denominator) -> recipZ ----
            pz = psS.tile([8, 512], F32, tag="small", name=f"psz{h}_{c}")
            for tk in range(TT):
                nc.tensor.matmul(
                    pz[:], ones8[:], ET[:, tk, _c(c)],
                    start=(tk == 0), stop=(tk == TT - 1),
                )
            nc.vector.reciprocal_approx_fast(recipZ[:, _c(c)], pz[:])

            # ---- qvRep [128(4x qk), T] ----
            pq = psA.tile([128, 512], F32, tag="acc", name=f"psq{h}_{c}")
            for kt in range(KT):
                nc.tensor.matmul(
                    pq[:], wqv[h][:, _t(kt)], xT[:, kt, _c(c)],
                    start=(kt == 0), stop=(kt == KT - 1),
                )
            if flags["bqv"]:
                nc.scalar.activation(
                    qvRep[:, _c(c)], pq[:],
                    mybir.ActivationFunctionType.Identity,
                    bias=bqv_sb[:, h : h + 1],
                )
            else:
                nc.scalar.copy(qvRep[:, _c(c)], pq[:])

            # ---- kvT (4 rules / psum via block-diag Wkv) + P-mul ----
            for g in range(2):
                pk = psA.tile([128, 512], F32, tag="acc", name=f"psk{h}_{c}_{g}")
                for rr in range(4):
                    r = g * 4 + rr
                    nc.tensor.matmul(
                        pk[:], wkvblk[:, rr], OTu[:, r, _c(c)],
                        start=(rr == 0), stop=(rr == 3),
                    )
                if flags["bkv"]:
                    tmp = scp.tile([128, 512], F32, tag="kvtmp", name=f"kvt{h}_{c}_{g}")
                    nc.vector.tensor_scalar_add(tmp[:], pk[:], bkv_sb[:, 0:1])
                    nc.vector.tensor_tensor(PP[:, g, _c(c)], tmp[:], qvRep[:, _c(c)], op=MUL)
                else:
                    nc.vector.tensor_tensor(PP[:, g, _c(c)], pk[:], qvRep[:, _c(c)], op=MUL)

            # ---- compU -> comp logits -> compE ----
            pc = psS.tile([8, 512], F32, tag="small", name=f"psc{h}_{c}")
            nc.tensor.matmul(pc[:], blkA[:], PP[:, 0, _c(c)], start=True, stop=False)
            nc.tensor.matmul(pc[:], blkB[:], PP[:, 1, _c(c)], start=False, stop=True)
            compL = scp.tile([8, 512], BF16, tag="compL", name=f"compL{h}_{c}", bufs=1)
            nc.vector.tensor_tensor(compL[:], pc[:], recipZ[:, _c(c)], op=MUL)
            nc.scalar.activation(compE[:, _c(c)], compL[:], EXP)

        w8_t = [None] * H
        w8d_t = [None] * H
        w8_t = [None] * H
        w8d_t = [None] * H

        def stage_b1c(h, c):
            recipZ, compE = recipZ_t[h], compE_t[h]
            # ---- CZ -> w8 = compE / (CZ * Z) ----
            if c == 0:
                w8_t[h] = hs.tile([8, T], BF16, tag="w8", name=f"w8{h}")
            w8 = w8_t[h]
            if True:
                pcz = psS.tile([8, 512], F32, tag="small", name=f"pscz{h}_{c}")
                nc.tensor.matmul(pcz[:], ones88[:], compE[:, _c(c)], start=True, stop=True)
                recipCZ = scp.tile([8, 512], F32, tag="recipCZ", name=f"rcz{h}_{c}", bufs=1)
                nc.vector.reciprocal_approx_fast(recipCZ[:], pcz[:])
                denom = scp.tile([8, 512], BF16, tag="denom", name=f"den{h}_{c}", bufs=1)
                nc.vector.tensor_tensor(denom[:], recipCZ[:], recipZ[:, _c(c)], op=MUL)
                nc.vector.tensor_tensor(w8[:, _c(c)], compE[:, _c(c)], denom[:], op=MUL)
            if c == 0:
                w8d_t[h] = w8dp.tile([8, T], BF16, tag="w8d", name=f"w8d{h}")
            nc.sync.dma_start(w8d_t[h][:, _c(c)], w8[:, _c(c)])

        def stage_b1(h):
            stage_b1c(h, 0)
            stage_b1c(h, 1)

        b2_acc = {}

        def stage_b2_pair(h, c, pair):
            # ---- broadcast w (PE select-matmul, rule pairs share a
            # 2-bank psum tile) + weighted sum over rules; one pair of
            # rules per call so the vector chain spreads across the
            # next head's stage-A hooks ----
            OTu, w8 = OTu_t[h], w8_t[h]
            for half in range(2):
                r = 2 * pair + half
                wrb = scp.tile([128, 512], BF16, tag=f"wrb{c}", name=f"wrb{h}_{c}_{r}")
                nc.sync.dma_start(
                    wrb[:], w8d_t[h][r : r + 1, _c(c)].to_broadcast([128, 512])
                )
                wr = wrb[:]
                if r == 0:
                    acc = scp.tile([128, 512], BF16, tag=f"acc{c}a", name=f"ac{h}_{c}_0", bufs=1)
                    nc.vector.tensor_tensor(acc[:], wr, OTu[:, r, _c(c)], op=MUL)
                    b2_acc[(h, c)] = acc
                else:
                    acc = b2_acc[(h, c)]
                    prod = scp.tile([128, 512], BF16, tag=f"prod{c}", name=f"pr{h}_{c}_{r}")
                    nc.vector.tensor_tensor(prod[:], wr, OTu[:, r, _c(c)], op=MUL)
                    if r < R - 1:
                        nacc = scp.tile([128, 512], BF16, tag=f"acc{c}{'b' if r % 2 else 'a'}", name=f"ac{h}_{c}_{r}", bufs=1)
                        nc.vector.tensor_tensor(nacc[:], acc[:], prod[:], op=ADD)
                        b2_acc[(h, c)] = nacc
                    else:
                        nc.vector.tensor_tensor(out2[:, h, _c(c)], acc[:], prod[:], op=ADD)

        def stage_b2(h, c):
            for pair in range(4):
                stage_b2_pair(h, c, pair)

        def merge_tiles(tts, pair_cb=None):
            # y = sum_h out2_h^T @ Wm_h for the given t-tiles
            for idx, tt in enumerate(tts):
                if pair_cb is not None and idx < 4:
                    pair_cb(idx)
                for c in range(NC2):
                    py = psA.tile([128, 512], F32, tag="acc", name=f"psy{tt}_{c}")
                    for h in range(H):
                        nc.tensor.matmul(
                            py[:], out2[:, h, _t(tt)], wm[:, h, _c(c)],
                            start=(h == 0), stop=(h == H - 1),
                        )
                    yt = scp.tile([128, 512], F32, tag="yt", name=f"yt{tt}_{c}")
                    nc.scalar.copy(yt[:], py[:])
                    nc.sync.dma_start(io["y"][_t(tt), _c(c)], yt[:])

        # ---- software-pipelined head loop: the previous head's
        # composition tail (B1 = w8 chain, B2 = broadcast + weighted
        # sum) is emitted interleaved into this head's stage A so the
        # PE never throttles on the vector chain. The startup DMA order
        # is critical-path aware (Sync triggers serialize at ~0.6us
        # each): wq first, then xT, then the rest of head 0's weights;
        # constants after those; wm (merge weights) deferred; later
        # heads prefetch from inside the previous head's stage A.
        wq[0] = hd.tile([128, D], BF16, tag="wq", name="wq0")
        nc.sync.dma_start(wq[0][:], io["WqP"][0])
        for kt in range(KT):
            nc.sync.dma_start(xT[:, kt], io["xT"][_t(kt), :])
        wk[0] = hd.tile([128, D], BF16, tag="wk", name="wk0")
        nc.sync.dma_start(wk[0][:], io["WkP"][0])
        wqv[0] = hd.tile([128, D], BF16, tag="wqv", name="wqv0")
        nc.sync.dma_start(wqv[0][:], io["WqvP"][0])
        wv[0] = hd.tile([128, KT, 1024], BF16, tag="wv", name="wv0", bufs=1)
        nc.sync.dma_start(wv[0][:], io["WvP"][0].rearrange("k (a rp) -> k a rp", a=KT))
        load_consts()

        def prefetch(h):
            if h == 2:
                nc.sync.dma_start(wm[:], io["WmP"].rearrange("k (h d) -> k h d", h=H))
            if h < H:
                load_weights(h)

        for h in range(H):
            if h >= 1:
                stage_a(
                    h,
                    after_qk=lambda hh=h - 1: stage_b1(hh),
                    step_cb=lambda i, hh=h - 1: stage_b2_pair(hh, i // 4, i % 4),
                )
            else:
                stage_a(h)
            prefetch(h + 1)
            if h < H - 1:
                post_v(h, 0)
                post_v(h, 1)
            else:
                # last head: pipeline its own composition tail against
                # chunk-1 work and the merge so the PE never drains
                post_v(h, 0)
                stage_b1c(h, 0)
                post_v(h, 1, pair_cb=lambda p: stage_b2_pair(h, 0, p))
                stage_b1c(h, 1)
                merge_tiles(range(0, 4), pair_cb=lambda p: stage_b2_pair(h, 1, p))
                merge_tiles(range(4, 8))



_CACHE = {}


def _build(flags_key):
    if flags_key in _CACHE:
        return _CACHE[flags_key]
    flags = dict(flags_key)
    nc = bacc.Bacc("TRN2", target_bir_lowering=False, debug=False, num_devices=NCORES)
    io = {}
    io["xT"] = nc.dram_tensor("xT", [D, T], BF16, kind="ExternalInput").ap()
    io["WqP"] = nc.dram_tensor("WqP", [H, 128, D], BF16, kind="ExternalInput").ap()
    io["WkP"] = nc.dram_tensor("WkP", [H, 128, D], BF16, kind="ExternalInput").ap()
    io["WqvP"] = nc.dram_tensor("WqvP", [H, 128, D], BF16, kind="ExternalInput").ap()
    io["WvP"] = nc.dram_tensor("WvP", [H, 128, KT * 1024], BF16, kind="ExternalInput").ap()
    io["WmP"] = nc.dram_tensor("WmP", [128, H * D], BF16, kind="ExternalInput").ap()
    io["c_wkvblk"] = nc.dram_tensor("c_wkvblk", [128, 512], BF16, kind="ExternalInput").ap()
    for bname, shape in (
        ("bq", [D]), ("bk", [D]), ("bv", [H * R * P]), ("bqv", [H * QK]), ("bkv", [QK]),
    ):
        if flags[bname]:
            dt = BF16 if bname == "bv" else F32
            io[bname] = nc.dram_tensor(bname, shape, dt, kind="ExternalInput").ap()
    io["c_ones8"] = nc.dram_tensor("c_ones8", [128, 8], BF16, kind="ExternalInput").ap()
    io["c_blkA"] = nc.dram_tensor("c_blkA", [128, 8], BF16, kind="ExternalInput").ap()
    io["c_blkB"] = nc.dram_tensor("c_blkB", [128, 8], BF16, kind="ExternalInput").ap()
    io["c_ones88"] = nc.dram_tensor("c_ones88", [8, 8], BF16, kind="ExternalInput").ap()
    io["c_sel8"] = nc.dram_tensor("c_sel8", [8, 1024], BF16, kind="ExternalInput").ap()
    if flags["bv"]:
        io["c_onesrow"] = nc.dram_tensor("c_onesrow", [1, 128], BF16, kind="ExternalInput").ap()
    io["y"] = nc.dram_tensor("y", [T, D], F32, kind="ExternalOutput").ap()

    with tile.TileContext(nc) as tc:
        build_kernel(tc, io, flags)
    nc.compile()
    _CACHE[flags_key] = (nc, flags)
    return _CACHE[flags_key]


def _wkvblk(Wkv):
    blk = np.zeros((128, 4, 128), np.float32)
    for g in range(4):
        blk[:, g, g * 32 : (g + 1) * 32] = Wkv
    return np.ascontiguousarray(blk.reshape(128, 512))


def _consts():
    ones8 = np.ones((128, 8), np.float32)
    blkA = np.zeros((128, 8), np.float32)
    blkB = np.zeros((128, 8), np.float32)
    for k in range(128):
        g = k // 32
        blkA[k, g] = 1.0
        blkB[k, 4 + g] = 1.0
    ones88 = np.ones((8, 8), np.float32)
    onesrow = np.ones((1, 128), np.float32)
    sel8 = np.zeros((8, 1024), np.float32)
    for m in range(1024):
        sel8[m // 128, m] = 1.0
    return {
        "c_ones8": ones8, "c_blkA": blkA, "c_blkB": blkB,
        "c_ones88": ones88, "c_onesrow": onesrow, "c_sel8": sel8,
    }


def _bf(a):
    return np.ascontiguousarray(a.astype(ml_dtypes.bfloat16))


def _pack_base(inp, flags):
    scale_q = np.float32(1.0 / np.sqrt(P))
    scale_qv = np.float32(1.0 / np.sqrt(QK))
    Wq_s = inp["Wq"] * scale_q
    Wqv_s = inp["Wqv"] * scale_qv
    # WqP[h,k,kt*128+m] = Wq_s[kt*128+k, h*128+m]
    WqP = Wq_s.reshape(KT, 128, H, 128).transpose(2, 1, 0, 3).reshape(H, 128, D)
    WkP = inp["Wk"].reshape(KT, 128, H, 128).transpose(2, 1, 0, 3).reshape(H, 128, D)
    # WqvP[h,k,kt*128+rep*32+j] = Wqv_s[kt*128+k, h*32+j]
    A = Wqv_s.reshape(KT, 128, H, QK).transpose(2, 1, 0, 3)  # [H,128,KT,QK]
    WqvP = np.broadcast_to(A[:, :, :, None, :], (H, 128, KT, 4, QK)).reshape(H, 128, D)
    # WvP[h,k,kt*1024+rp] = Wv[kt*128+k, h*1024+rp]
    WvP = inp["Wv"].reshape(KT, 128, H, 1024).transpose(2, 1, 0, 3).reshape(H, 128, KT * 1024)
    # WmP[k, h*1024+d] = Wm[h*128+k, d]
    WmP = inp["Wm"].reshape(H, 128, D).transpose(1, 0, 2).reshape(128, H * D)
    consts = _consts()
    base = {
        "WqP": _bf(WqP), "WkP": _bf(WkP), "WqvP": _bf(WqvP),
        "WvP": _bf(WvP), "WmP": _bf(WmP),
        "c_wkvblk": _bf(_wkvblk(inp["Wkv"])),
        "c_ones8": _bf(consts["c_ones8"]), "c_blkA": _bf(consts["c_blkA"]),
        "c_blkB": _bf(consts["c_blkB"]), "c_ones88": _bf(consts["c_ones88"]),
        "c_sel8": _bf(consts["c_sel8"]),
    }
    if flags["bq"]:
        base["bq"] = np.ascontiguousarray(inp["bq"] * scale_q)
    if flags["bk"]:
        base["bk"] = np.ascontiguousarray(inp["bk"])
    if flags["bqv"]:
        base["bqv"] = np.ascontiguousarray(inp["bqv"] * scale_qv)
    if flags["bv"]:
        base["bv"] = _bf(inp["bv"])
        base["c_onesrow"] = _bf(consts["c_onesrow"])
    if flags["bkv"]:
        base["bkv"] = np.ascontiguousarray(inp["bkv"])
    return base


def _run(inputs, trace=False):
    inp = {k: np.ascontiguousarray(np.asarray(v, dtype=np.float32)) for k, v in inputs.items()}
    flags = {b: bool(np.any(inp[b])) for b in ("bq", "bk", "bv", "bqv", "bkv")}
    flags_key = tuple(sorted(flags.items()))
    nc, flags = _build(flags_key)
    base = _pack_base(inp, flags)
    in_maps = []
    for c in range(NCORES):
        m = dict(base)
        m["xT"] = _bf(inp["x"][c].T)
        in_maps.append(m)
    res = run_bass_kernel_spmd(nc, in_maps, list(range(NCORES)), trace=trace)
    out = np.stack([res.results[c]["y"] for c in range(NCORES)], axis=0)
    return out, res


def kernel(**inputs):
    out, _ = _run(inputs, trace=False)
    return out


def run_traced(inputs):
    """Like kernel() but with NTFF tracing; returns (out, BassKernelResults)."""
    return _run(inputs, trace=True)


# revision 17
# speedup vs baseline: 1.0581x; 1.0520x over previous
"""Trainium2 Bass kernel for CompositionalAttentionBase.

Problem (per batch element b, reference semantics):
  q = (x @ Wq + bq)  -> [T,H,P] * 1/sqrt(P)
  k = (x @ Wk + bk)  -> [T,H,P]
  v = (x @ Wv + bv)  -> [T,H,R,P]
  score = softmax(q k^T) per head            [H,Tq,Tk]
  out   = score @ v per (head, rule)         [T,H,R,P]
  q_v = (x @ Wqv + bqv)/sqrt(QK)             [T,H,QK]
  k_v = out @ Wkv + bkv                      [T,H,R,QK]
  comp = softmax_r(q_v . k_v)                [T,H,R]
  out2 = sum_r comp * out                    [T,H,P]
  y = out2.reshape(T,D) @ Wm

Sharding: pure data-parallel over batch. B == n_cores == 8, so each
NeuronCore computes one full batch element; no collectives at all.

v2 design notes (vs the fp32r baseline):
  - Everything on the PE is bf16 (PSUM accumulation stays fp32). At
    N=512 the matmul streams at 1 col/cycle for both fp32r and bf16,
    but bf16 enables fast-weight-load (64-cycle LDWEIGHTS, hidden
    behind the 512-cycle matmul) and halves all SBUF/DMA/evacuation
    traffic.
  - x is pre-transposed on the host (xT [D,T]) and all weights are
    pre-packed host-side into per-head contiguous layouts, so every
    weight load is one large DMA and the kernel does zero PE
    transposes.
  - The per-head program is split into stage A (projections, scores,
    V, retrieval OTu, composition logits) and stage B (composition
    softmax tail + rule-weighted sum). B(h-1) is emitted after A(h),
    so the PE never waits on the vector-engine softmax chain at a
    head boundary.
  - The rule-weighted sum uses a contiguous multiply/add chain on
    DVE instead of one big strided tensor_reduce (which measured
    ~15us per head in the baseline trace).

Per-core dataflow (head-by-head; all contractions natural TensorE
matmuls, scores kept unnormalized with 1/Z folded into the final
composition weights):
  qT_h = Wq_h^T @ xT        [P,T]     (Wq pre-scaled by 1/sqrt(P))
  kT_h = Wk_h^T @ xT        [P,T]
  ET   = exp(kT^T q-slices) [Tk,Tq]
  V_h  = xT^T @ Wv_h        [Tk,R*P]
  OTu_r = V_r^T @ ET        [P,Tq]    (unnormalized attention out)
  ZRep8 = ones8^T @ ET      [8,Tq]    -> recipZ
  qvT  = Wqv_h^T @ xT (4x row-replicated) [4*QK,T]
  kvT  = Wkv^T @ OTu_r (block-diag, 4 rules/psum) [4*QK,Tq]
  compU = blockdiag-sums of (kvT * qvT)   [8,Tq]
  compE = exp(compU / Z);  w = compE / (CZ * Z)
  out2_h = sum_r OTu_r * broadcast(w_r)   [P,T]
  y = sum_h out2_h^T @ Wm_h               [T,D]
"""

import numpy as np
import ml_dtypes

import concourse.bass as bass
import concourse.tile as tile
from concourse import bacc, mybir
from concourse.bass_utils import run_bass_kernel_spmd

B, T, D, H, R, QK = 8, 1024, 1024, 8, 8, 32
P = D // H  # 128
NCORES = 8
TT = T // 128  # 8 t-tiles
KT = D // 128  # 8 contraction tiles for D
NC2 = T // 512  # 2 free-dim chunks of 512 over T
F32 = mybir.dt.float32
BF16 = mybir.dt.bfloat16
EXP = mybir.ActivationFunctionType.Exp
MUL = mybir.AluOpType.mult
ADD = mybir.AluOpType.add


def _c(c):  # 512-chunk slice
    return slice(c * 512, (c + 1) * 512)


def _t(i):  # 128-tile slice
    return slice(i * 128, (i + 1) * 128)


def build_kernel(tc, io, flags):
    nc = tc.nc

    with (
        nc.allow_low_precision(reason="bf16 intermediates; end-to-end precision validated vs reference"),
        tc.tile_pool(name="cst", bufs=1) as cst,
        tc.tile_pool(name="per", bufs=1) as per,
        tc.tile_pool(name="hd", bufs=2) as hd,     # double-buffered per-head
        tc.tile_pool(name="hs", bufs=1) as hs,     # single-buffered per-head
        tc.tile_pool(name="sc", bufs=2) as scp,    # small vector scratch
        tc.tile_pool(name="w8d", bufs=2, space="DRAM") as w8dp,
        tc.tile_pool(name="psA", bufs=6, space="PSUM") as psA,
        tc.tile_pool(name="psS", bufs=2, space="PSUM") as psS,
    ):
        # ---- constants (DMAs emitted after the critical first-head
        # weight loads; none is read before the Z stage of head 0) ----
        ones8 = cst.tile([128, 8], BF16, name="ones8")
        blkA = cst.tile([128, 8], BF16, name="blkA")
        blkB = cst.tile([128, 8], BF16, name="blkB")
        ones88 = cst.tile([8, 8], BF16, name="ones88")
        sel8 = cst.tile([8, 1024], BF16, name="sel8")
        wkvblk = cst.tile([128, 4, 128], BF16, name="wkvblk")

        def load_consts():
            nc.sync.dma_start(ones8[:], io["c_ones8"])
            nc.sync.dma_start(blkA[:], io["c_blkA"])
            nc.sync.dma_start(blkB[:], io["c_blkB"])
            nc.sync.dma_start(ones88[:], io["c_ones88"])
            nc.sync.dma_start(sel8[:], io["c_sel8"])
            nc.sync.dma_start(wkvblk[:], io["c_wkvblk"].rearrange("p (g m) -> p g m", g=4))
        if flags["bq"]:
            bq_sb = cst.tile([128, 8], F32, name="bq_sb")
            nc.sync.dma_start(bq_sb[:], io["bq"].rearrange("(h p) -> p h", p=128))
        if flags["bk"]:
            bk_sb = cst.tile([128, 8], F32, name="bk_sb")
            nc.sync.dma_start(bk_sb[:], io["bk"].rearrange("(h p) -> p h", p=128))
        if flags["bqv"]:
            bqv_sb = cst.tile([128, 8], F32, name="bqv_sb")
            nc.sync.dma_start(
                bqv_sb[:],
                io["bqv"].rearrange("(h q) -> q h", q=32).to_broadcast([4, 32, 8]).rearrange("r q h -> (r q) h"),
            )
        if flags["bv"]:
            onesrow = cst.tile([1, 128], BF16, name="onesrow")
            nc.sync.dma_start(onesrow[:], io["c_onesrow"])
        if flags["bkv"]:
            bkv_sb = cst.tile([128, 1], F32, name="bkv_sb")
            nc.sync.dma_start(
                bkv_sb[:], io["bkv"].rearrange("(o q) -> q o", o=1).to_broadcast([4, 32, 1]).rearrange("r q o -> (r q) o")
            )

        # ---- persistent tiles (wm DMA deferred; see head loop) ----
        xT = per.tile([128, KT, T], BF16, name="xT")
        wm = per.tile([128, H, D], BF16, name="wm")
        out2 = per.tile([128, H, T], BF16, name="out2")

        # ---- per-head weight loads (prefetched one head ahead) ----
        wq = [None] * H
        wk = [None] * H
        wqv = [None] * H
        wv = [None] * H

        def load_weights(h):
            wq[h] = hd.tile([128, D], BF16, tag="wq", name=f"wq{h}")
            nc.sync.dma_start(wq[h][:], io["WqP"][h])
            wk[h] = hd.tile([128, D], BF16, tag="wk", name=f"wk{h}")
            nc.sync.dma_start(wk[h][:], io["WkP"][h])
            wqv[h] = hd.tile([128, D], BF16, tag="wqv", name=f"wqv{h}")
            nc.sync.dma_start(wqv[h][:], io["WqvP"][h])
            wv[h] = hd.tile([128, KT, 1024], BF16, tag="wv", name=f"wv{h}", bufs=1)
            nc.sync.dma_start(wv[h][:], io["WvP"][h].rearrange("k (a rp) -> k a rp", a=KT))

        # per-head state handed between stages
        OTu_t = [None] * H
        recipZ_t = [None] * H
        compE_t = [None] * H
        ET_t = [None] * H
        V_t = [None] * H

        def stage_a(h, after_qk=None, step_cb=None):
            # ---- qT / kT (one 2-bank psum + one 1024-wide evac each) ----
            qT = hd.tile([128, T], BF16, tag="qT", name=f"qT{h}", bufs=1)
            kT = hd.tile([128, T], BF16, tag="kT", name=f"kT{h}", bufs=1)
            for dst, w, bflag in (
                (qT, wq[h], flags["bq"]),
                (kT, wk[h], flags["bk"]),
            ):
                for c in range(NC2):
                    ps = psA.tile([128, 512], F32, tag="acc", name=f"psqk{h}_{c}")
                    for kt in range(KT):
                        nc.tensor.matmul(
                            ps[:], w[:, _t(kt)], xT[:, kt, _c(c)],
                            start=(kt == 0), stop=(kt == KT - 1),
                        )
                    if bflag:
                        nc.scalar.activation(
                            dst[:, _c(c)], ps[:],
                            mybir.ActivationFunctionType.Identity,
                            bias=(bq_sb if dst is qT else bk_sb)[:, h : h + 1],
                        )
                    else:
                        nc.scalar.copy(dst[:, _c(c)], ps[:])

            if after_qk is not None:
                after_qk()

            # ---- ET = exp(scores^T) interleaved with V so the PE covers
            # the scalar-engine exp evacuations ----
            ET = hs.tile([128, TT, T], BF16, tag="ET", name=f"ET{h}")
            V = hs.tile([128, TT, 1024], BF16, tag="V", name=f"V{h}")
            ET_t[h], V_t[h] = ET, V
            for i in range(TT):
                # ET group (tk=i): 2 matmuls + exps
                for c in range(NC2):
                    pe = psA.tile([128, 512], F32, tag="acc", name=f"pse{h}_{i}_{c}")
                    nc.tensor.matmul(pe[:], kT[:, _t(i)], qT[:, _c(c)], start=True, stop=True)
                    nc.scalar.activation(ET[:, i, _c(c)], pe[:], EXP)
                # V group (tt=i)
                for c in range(2):
                    pv = psA.tile([128, 512], F32, tag="acc", name=f"psv{h}_{i}_{c}")
                    for kt in range(KT):
                        nc.tensor.matmul(
                            pv[:], xT[:, kt, _t(i)], wv[h][:, kt, _c(c)],
                            start=(kt == 0), stop=(kt == KT - 1 and not flags["bv"]),
                        )
                    if flags["bv"]:
                        if c == 0:
                            bv_t = scp.tile([1, 1024], BF16, tag="bv", name=f"bv{h}_{i}")
                            nc.sync.dma_start(bv_t[:], io["bv"][None, h * 1024 : (h + 1) * 1024])
                        nc.tensor.matmul(pv[:], onesrow[:], bv_t[:, _c(c)], start=False, stop=True)
                    nc.vector.tensor_copy(V[:, i, _c(c)], pv[:])
                if step_cb is not None:
                    step_cb(i)

            # post-V per-chunk work is emitted by the head loop (post_v)

        qvRep_t = [None] * H
        PP_t = [None] * H

        def post_v(h, c, pair_cb=None):
            """OTu/Z/qv/kv/compU for one 512-wide t-chunk, with optional
            B2-pair hooks interleaved between OTu rule tiles."""
            ET, V = ET_t[h], V_t[h]
            if c == 0:
                OTu_t[h] = hd.tile([128, R, T], BF16, tag="OTu", name=f"OTu{h}")
                recipZ_t[h] = hd.tile([8, T], F32, tag="recipZ", name=f"recipZ{h}")
                qvRep_t[h] = hs.tile([128, T], BF16, tag="qvRep", name=f"qvRep{h}")
                PP_t[h] = hs.tile([128, 2, T], BF16, tag="PP", name=f"PP{h}")
                compE_t[h] = hd.tile([8, T], BF16, tag="compE", name=f"compE{h}")
            OTu, recipZ, qvRep, PP, compE = (
                OTu_t[h], recipZ_t[h], qvRep_t[h], PP_t[h], compE_t[h])

            # ---- OTu_r = V_r^T @ ET  [128(p), R, T(q)] ----
            for r in range(R):
                po = psA.tile([128, 512], F32, tag="acc", name=f"pso{h}_{r}_{c}")
                for tk in range(TT):
                    nc.tensor.matmul(
                        po[:], V[:, tk, _t(r)], ET[:, tk, _c(c)],
                        start=(tk == 0), stop=(tk == TT - 1),
                    )
                nc.scalar.copy(OTu[:, r, _c(c)], po[:])
                if pair_cb is not None and r % 2 == 1:
                    pair_cb((r - 1) // 2)

            # ---- Z (softmax denominator) -> recipZ ----
            pz = psS.tile([8, 512], F32, tag="small", name=f"psz{h}_{c}")
            for tk in range(TT):
                nc.tensor.matmul(
                    pz[:], ones8[:], ET[:, tk, _c(c)],
                    start=(tk == 0), stop=(tk == TT - 1),
                )
            nc.vector.reciprocal_approx_fast(recipZ[:, _c(c)], pz[:])

            # ---- qvRep [128(4x qk), T] ----
            pq = psA.tile([128, 512], F32, tag="acc", name=f"psq{h}_{c}")
            for kt in range(KT):
                nc.tensor.matmul(
                    pq[:], wqv[h][:, _t(kt)], xT[:, kt, _c(c)],
                    start=(kt == 0), stop=(kt == KT - 1),
                )
            if flags["bqv"]:
                nc.scalar.activation(
                    qvRep[:, _c(c)], pq[:],
                    mybir.ActivationFunctionType.Identity,
                    bias=bqv_sb[:, h : h + 1],
                )
            else:
                nc.scalar.copy(qvRep[:, _c(c)], pq[:])

            # ---- kvT (4 rules / psum via block-diag Wkv) + P-mul ----
            for g in range(2):
                pk = psA.tile([128, 512], F32, tag="acc", name=f"psk{h}_{c}_{g}")
                for rr in range(4):
                    r = g * 4 + rr
                    nc.tensor.matmul(
                        pk[:], wkvblk[:, rr], OTu[:, r, _c(c)],
                        start=(rr == 0), stop=(rr == 3),
                    )
                if flags["bkv"]:
                    tmp = scp.tile([128, 512], F32, tag="kvtmp", name=f"kvt{h}_{c}_{g}")
                    nc.vector.tensor_scalar_add(tmp[:], pk[:], bkv_sb[:, 0:1])
                    nc.vector.tensor_tensor(PP[:, g, _c(c)], tmp[:], qvRep[:, _c(c)], op=MUL)
                else:
                    nc.vector.tensor_tensor(PP[:, g, _c(c)], pk[:], qvRep[:, _c(c)], op=MUL)

            # ---- compU -> comp logits -> compE ----
            pc = psS.tile([8, 512], F32, tag="small", name=f"psc{h}_{c}")
            nc.tensor.matmul(pc[:], blkA[:], PP[:, 0, _c(c)], start=True, stop=False)
            nc.tensor.matmul(pc[:], blkB[:], PP[:, 1, _c(c)], start=False, stop=True)
            compL = scp.tile([8, 512], BF16, tag="compL", name=f"compL{h}_{c}", bufs=1)
            nc.vector.tensor_tensor(compL[:], pc[:], recipZ[:, _c(c)], op=MUL)
            nc.scalar.activation(compE[:, _c(c)], compL[:], EXP)

        w8_t = [None] * H
        w8d_t = [None] * H
        w8_t = [None] * H
        w8d_t = [None] * H

        def stage_b1c(h, c):
            recipZ, compE = recipZ_t[h], compE_t[h]
            # ---- CZ -> w8 = compE / (CZ * Z) ----
            if c == 0:
                w8_t[h] = hs.tile([8, T], BF16, tag="w8", name=f"w8{h}")
            w8 = w8_t[h]
            if True:
                pcz = psS.tile([8, 512], F32, tag="small", name=f"pscz{h}_{c}")
                nc.tensor.matmul(pcz[:], ones88[:], compE[:, _c(c)], start=True, stop=True)
                recipCZ = scp.tile([8, 512], F32, tag="recipCZ", name=f"rcz{h}_{c}", bufs=1)
                nc.vector.reciprocal_approx_fast(recipCZ[:], pcz[:])
                denom = scp.tile([8, 512], BF16, tag="denom", name=f"den{h}_{c}", bufs=1)
                nc.vector.tensor_tensor(denom[:], recipCZ[:], recipZ[:, _c(c)], op=MUL)
                nc.vector.tensor_tensor(w8[:, _c(c)], compE[:, _c(c)], denom[:], op=MUL)
            if c == 0:
                w8d_t[h] = w8dp.tile([8, T], BF16, tag="w8d", name=f"w8d{h}")
            nc.sync.dma_start(w8d_t[h][:, _c(c)], w8[:, _c(c)])

        def stage_b1(h):
            stage_b1c(h, 0)
            stage_b1c(h, 1)

        b2_acc = {}

        def stage_b2_pair(h, c, pair, use_pe=False):
            # ---- broadcast w8 rows to 128 partitions: via DMA from the
            # DRAM copy (cheap, off the PE) for pipelined heads, via PE
            # select-matmul for the latency-critical last head ----
            OTu, w8 = OTu_t[h], w8_t[h]
            for half in range(2):
                r = 2 * pair + half
                if use_pe:
                    wrp = psA.tile([128, 512], F32, tag="acc", name=f"wrp{h}_{c}_{r}")
                    nc.tensor.matmul(wrp[:], sel8[:, _t(r)], w8[:, _c(c)], start=True, stop=True)
                    wr = wrp[:]
                else:
                    wrb = scp.tile([128, 512], BF16, tag=f"wrb{c}", name=f"wrb{h}_{c}_{r}", bufs=4)
                    nc.sync.dma_start(
                        wrb[:], w8d_t[h][r : r + 1, _c(c)].to_broadcast([128, 512])
                    )
                    wr = wrb[:]
                if r == 0:
                    acc = scp.tile([128, 512], BF16, tag=f"acc{c}a", name=f"ac{h}_{c}_0", bufs=1)
                    nc.vector.tensor_tensor(acc[:], wr, OTu[:, r, _c(c)], op=MUL)
                    b2_acc[(h, c)] = acc
                else:
                    acc = b2_acc[(h, c)]
                    prod = scp.tile([128, 512], BF16, tag=f"prod{c}", name=f"pr{h}_{c}_{r}")
                    nc.vector.tensor_tensor(prod[:], wr, OTu[:, r, _c(c)], op=MUL)
                    if r < R - 1:
                        nacc = scp.tile([128, 512], BF16, tag=f"acc{c}{'b' if r % 2 else 'a'}", name=f"ac{h}_{c}_{r}", bufs=1)
                        nc.vector.tensor_tensor(nacc[:], acc[:], prod[:], op=ADD)
                        b2_acc[(h, c)] = nacc
                    else:
                        nc.vector.tensor_tensor(out2[:, h, _c(c)], acc[:], prod[:], op=ADD)

        def stage_b2(h, c):
            for pair in range(4):
                stage_b2_pair(h, c, pair)

        def merge_tiles(tts, pair_cb=None):
            # y = sum_h out2_h^T @ Wm_h for the given t-tiles
            for idx, tt in enumerate(tts):
                if pair_cb is not None and idx < 4:
                    pair_cb(idx)
                for c in range(NC2):
                    py = psA.tile([128, 512], F32, tag="acc", name=f"psy{tt}_{c}")
                    for h in range(H):
                        nc.tensor.matmul(
                            py[:], out2[:, h, _t(tt)], wm[:, h, _c(c)],
                            start=(h == 0), stop=(h == H - 1),
                        )
                    yt = scp.tile([128, 512], F32, tag="yt", name=f"yt{tt}_{c}")
                    nc.scalar.copy(yt[:], py[:])
                    nc.sync.dma_start(io["y"][_t(tt), _c(c)], yt[:])

        # ---- software-pipelined head loop: the previous head's
        # composition tail (B1 = w8 chain, B2 = broadcast + weighted
        # sum) is emitted interleaved into this head's stage A so the
        # PE never throttles on the vector chain. The startup DMA order
        # is critical-path aware (Sync triggers serialize at ~0.6us
        # each): wq first, then xT, then the rest of head 0's weights;
        # constants after those; wm (merge weights) deferred; later
        # heads prefetch from inside the previous head's stage A.
        wq[0] = hd.tile([128, D], BF16, tag="wq", name="wq0")
        nc.sync.dma_start(wq[0][:], io["WqP"][0])
        for kt in range(KT):
            nc.sync.dma_start(xT[:, kt], io["xT"][_t(kt), :])
        wk[0] = hd.tile([128, D], BF16, tag="wk", name="wk0")
        nc.sync.dma_start(wk[0][:], io["WkP"][0])
        wqv[0] = hd.tile([128, D], BF16, tag="wqv", name="wqv0")
        nc.sync.dma_start(wqv[0][:], io["WqvP"][0])
        wv[0] = hd.tile([128, KT, 1024], BF16, tag="wv", name="wv0", bufs=1)
        nc.sync.dma_start(wv[0][:], io["WvP"][0].rearrange("k (a rp) -> k a rp", a=KT))
        load_consts()

        def prefetch(h):
            if h == 2:
                nc.sync.dma_start(wm[:], io["WmP"].rearrange("k (h d) -> k h d", h=H))
            if h < H:
                load_weights(h)

        for h in range(H):
            if h >= 1:
                stage_a(
                    h,
                    after_qk=lambda hh=h - 1: stage_b1(hh),
                    step_cb=lambda i, hh=h - 1: stage_b2_pair(hh, i // 4, i % 4),
                )
            else:
                stage_a(h)
            prefetch(h + 1)
            if h < H - 1:
                post_v(h, 0)
                post_v(h, 1)
            else:
                # last head: pipeline its own composition tail against
                # chunk-1 work and the merge so the PE never drains
                post_v(h, 0)
                stage_b1c(h, 0)
                post_v(h, 1, pair_cb=lambda p: stage_b2_pair(h, 0, p, use_pe=True))
                stage_b1c(h, 1)
                merge_tiles(range(0, 4), pair_cb=lambda p: stage_b2_pair(h, 1, p, use_pe=True))
                merge_tiles(range(4, 8))



_CACHE = {}


def _build(flags_key):
    if flags_key in _CACHE:
        return _CACHE[flags_key]
    flags = dict(flags_key)
    nc = bacc.Bacc("TRN2", target_bir_lowering=False, debug=False, num_devices=NCORES)
    io = {}
    io["xT"] = nc.dram_tensor("xT", [D, T], BF16, kind="ExternalInput").ap()
    io["WqP"] = nc.dram_tensor("WqP", [H, 128, D], BF16, kind="ExternalInput").ap()
    io["WkP"] = nc.dram_tensor("WkP", [H, 128, D], BF16, kind="ExternalInput").ap()
    io["WqvP"] = nc.dram_tensor("WqvP", [H, 128, D], BF16, kind="ExternalInput").ap()
    io["WvP"] = nc.dram_tensor("WvP", [H, 128, KT * 1024], BF16, kind="ExternalInput").ap()
    io["WmP"] = nc.dram_tensor("WmP", [128, H * D], BF16, kind="ExternalInput").ap()
    io["c_wkvblk"] = nc.dram_tensor("c_wkvblk", [128, 512], BF16, kind="ExternalInput").ap()
    for bname, shape in (
        ("bq", [D]), ("bk", [D]), ("bv", [H * R * P]), ("bqv", [H * QK]), ("bkv", [QK]),
    ):
        if flags[bname]:
            dt = BF16 if bname == "bv" else F32
            io[bname] = nc.dram_tensor(bname, shape, dt, kind="ExternalInput").ap()
    io["c_ones8"] = nc.dram_tensor("c_ones8", [128, 8], BF16, kind="ExternalInput").ap()
    io["c_blkA"] = nc.dram_tensor("c_blkA", [128, 8], BF16, kind="ExternalInput").ap()
    io["c_blkB"] = nc.dram_tensor("c_blkB", [128, 8], BF16, kind="ExternalInput").ap()
    io["c_ones88"] = nc.dram_tensor("c_ones88", [8, 8], BF16, kind="ExternalInput").ap()
    io["c_sel8"] = nc.dram_tensor("c_sel8", [8, 1024], BF16, kind="ExternalInput").ap()
    if flags["bv"]:
        io["c_onesrow"] = nc.dram_tensor("c_onesrow", [1, 128], BF16, kind="ExternalInput").ap()
    io["y"] = nc.dram_tensor("y", [T, D], F32, kind="ExternalOutput").ap()

    with tile.TileContext(nc) as tc:
        build_kernel(tc, io, flags)
    nc.compile()
    _CACHE[flags_key] = (nc, flags)
    return _CACHE[flags_key]


def _wkvblk(Wkv):
    blk = np.zeros((128, 4, 128), np.float32)
    for g in range(4):
        blk[:, g, g * 32 : (g + 1) * 32] = Wkv
    return np.ascontiguousarray(blk.reshape(128, 512))


def _consts():
    ones8 = np.ones((128, 8), np.float32)
    blkA = np.zeros((128, 8), np.float32)
    blkB = np.zeros((128, 8), np.float32)
    for k in range(128):
        g = k // 32
        blkA[k, g] = 1.0
        blkB[k, 4 + g] = 1.0
    ones88 = np.ones((8, 8), np.float32)
    onesrow = np.ones((1, 128), np.float32)
    sel8 = np.zeros((8, 1024), np.float32)
    for m in range(1024):
        sel8[m // 128, m] = 1.0
    return {
        "c_ones8": ones8, "c_blkA": blkA, "c_blkB": blkB,
        "c_ones88": ones88, "c_onesrow": onesrow, "c_sel8": sel8,
    }


def _bf(a):
    return np.ascontiguousarray(a.astype(ml_dtypes.bfloat16))


def _pack_base(inp, flags):
    scale_q = np.float32(1.0 / np.sqrt(P))
    scale_qv = np.float32(1.0 / np.sqrt(QK))
    Wq_s = inp["Wq"] * scale_q
    Wqv_s = inp["Wqv"] * scale_qv
    # WqP[h,k,kt*128+m] = Wq_s[kt*128+k, h*128+m]
    WqP = Wq_s.reshape(KT, 128, H, 128).transpose(2, 1, 0, 3).reshape(H, 128, D)
    WkP = inp["Wk"].reshape(KT, 128, H, 128).transpose(2, 1, 0, 3).reshape(H, 128, D)
    # WqvP[h,k,kt*128+rep*32+j] = Wqv_s[kt*128+k, h*32+j]
    A = Wqv_s.reshape(KT, 128, H, QK).transpose(2, 1, 0, 3)  # [H,128,KT,QK]
    WqvP = np.broadcast_to(A[:, :, :, None, :], (H, 128, KT, 4, QK)).reshape(H, 128, D)
    # WvP[h,k,kt*1024+rp] = Wv[kt*128+k, h*1024+rp]
    WvP = inp["Wv"].reshape(KT, 128, H, 1024).transpose(2, 1, 0, 3).reshape(H, 128, KT * 1024)
    # WmP[k, h*1024+d] = Wm[h*128+k, d]
    WmP = inp["Wm"].reshape(H, 128, D).transpose(1, 0, 2).reshape(128, H * D)
    consts = _consts()
    base = {
        "WqP": _bf(WqP), "WkP": _bf(WkP), "WqvP": _bf(WqvP),
        "WvP": _bf(WvP), "WmP": _bf(WmP),
        "c_wkvblk": _bf(_wkvblk(inp["Wkv"])),
        "c_ones8": _bf(consts["c_ones8"]), "c_blkA": _bf(consts["c_blkA"]),
        "c_blkB": _bf(consts["c_blkB"]), "c_ones88": _bf(consts["c_ones88"]),
        "c_sel8": _bf(consts["c_sel8"]),
    }
    if flags["bq"]:
        base["bq"] = np.ascontiguousarray(inp["bq"] * scale_q)
    if flags["bk"]:
        base["bk"] = np.ascontiguousarray(inp["bk"])
    if flags["bqv"]:
        base["bqv"] = np.ascontiguousarray(inp["bqv"] * scale_qv)
    if flags["bv"]:
        base["bv"] = _bf(inp["bv"])
        base["c_onesrow"] = _bf(consts["c_onesrow"])
    if flags["bkv"]:
        base["bkv"] = np.ascontiguousarray(inp["bkv"])
    return base


def _run(inputs, trace=False):
    inp = {k: np.ascontiguousarray(np.asarray(v, dtype=np.float32)) for k, v in inputs.items()}
    flags = {b: bool(np.any(inp[b])) for b in ("bq", "bk", "bv", "bqv", "bkv")}
    flags_key = tuple(sorted(flags.items()))
    nc, flags = _build(flags_key)
    base = _pack_base(inp, flags)
    in_maps = []
    for c in range(NCORES):
        m = dict(base)
        m["xT"] = _bf(inp["x"][c].T)
        in_maps.append(m)
    res = run_bass_kernel_spmd(nc, in_maps, list(range(NCORES)), trace=trace)
    out = np.stack([res.results[c]["y"] for c in range(NCORES)], axis=0)
    return out, res


def kernel(**inputs):
    out, _ = _run(inputs, trace=False)
    return out


def run_traced(inputs):
    """Like kernel() but with NTFF tracing; returns (out, BassKernelResults)."""
    return _run(inputs, trace=True)
